# revision 1
# baseline (speedup 1.0000x reference)
"""NT-Xent loss kernel for Trainium2, 8-core SPMD.

Math: with p = cat(z_i, z_j) [8192, 64], pn = p / max(||p||, 1e-8),
sim = 2 * pn @ pn.T (TEMP=0.5), the reference's gather-based losses reduce to
  loss1 = mean_r( log(sum_{c != r} exp(sim[r,c])) - pos_r )
  loss2 = mean_r( log(exp(pos_r) + sum_{c != t_r} exp(probs[r,c])) - pos_r )
where pos_r = sim[r, (r+N) % 2N].  sim entries lie in [-2, 2], so the exp
never overflows and no max-shift pass is needed.  The huge neg_idx input is a
fixed structured mask (drop self + positive) and never needs to be read.

Sharding: row-parallel.  Each of the 8 cores gets 1024 rows of the sim matrix,
computes sum_c exp(2 * pn_shard @ pn.T) against the full all-rows pn (computed
redundantly on every core from the full p), plus its rows' pos/diag terms and
the probs part, and emits two partial sums.  Host adds the 8 partials.
"""

import numpy as np

import concourse.bass as bass
import concourse.bacc as bacc
import concourse.tile as tile
from concourse import mybir
from concourse.masks import make_identity
from concourse.bass_utils import run_bass_kernel_spmd

N = 4096
D = 64
M = 2 * N            # 8192 rows of sim
NCORES = 8
R = M // NCORES      # 1024 rows per core
NT = M // 128        # 64 row-tiles of the full p
NS = R // 128        # 8 row-tiles of a shard
NCLS = 10
INV_TEMP = 2.0       # 1 / 0.5
F32 = mybir.dt.float32
BF16 = mybir.dt.bfloat16

# bf16 matmul for the sim slab: 4x PE throughput, 2x moving-dim. pos/diag
# stay fp32 (computed on DVE), and per-row errors average out over 8192 rows.
import os
USE_BF16_MM = os.environ.get("NTX_BF16", "1") == "1"
USE_GPSIMD = os.environ.get("NTX_GPS", "0") == "1"
N_ACC_ENV = int(os.environ.get("NTX_NACC", "28"))
NEWTON_ITERS = int(os.environ.get("NTX_NEWT", "2"))
BENCH_REPS = int(os.environ.get("NTX_REPS", "0"))  # 0 = no loop

AF = mybir.ActivationFunctionType
ALU = mybir.AluOpType


def _emit_rsqrt(nc, pool, n2, nchunk, eng=None):
    """inv = 1/max(sqrt(n2), 1e-8), entirely on DVE: quake-style magic
    constant seed + 3 Newton steps (ACT stays exclusively on Exp/Ln, so the
    activation table never thrashes).  Newton converges the seed's 3.4% max
    error to below fp32 rounding."""
    if eng is None:
        eng = nc.vector
    I32 = mybir.dt.int32
    inv = pool.tile([128, nchunk], F32, tag="rs_inv")
    # seed: bits = 0x5f3759df - (bits(n2) >> 1)   (shift and arith must be
    # separate instructions -- walrus rejects mixed-class op0/op1)
    eng.tensor_scalar(inv.bitcast(I32), n2.bitcast(I32), 1, None,
                      ALU.arith_shift_right)
    eng.tensor_scalar(inv.bitcast(I32), inv.bitcast(I32), -1, 0x5F3759DF,
                      ALU.mult, ALU.add)
    t2 = pool.tile([128, nchunk], F32, tag="rs_t2")
    for _ in range(NEWTON_ITERS):
        # y' = y * (1.5 - 0.5 * n2 * y^2)
        eng.tensor_mul(t2, inv, inv)
        eng.tensor_mul(t2, t2, n2)
        eng.tensor_scalar(t2, t2, -0.5, 1.5, ALU.mult, ALU.add)
        eng.tensor_mul(inv, inv, t2)
    eng.tensor_scalar_min(inv, inv, 1e8)
    return inv


def _emit_normalize(nc, pool, raw, ntiles, tag, eng=None):
    """raw: [128, ntiles, 64] -> pn (same shape), rows normalized.

    The row scale is applied per 64-wide chunk with tensor_scalar_mul and a
    per-partition scalar AP (free-dim-broadcast APs with step 0 silently
    corrupt on HW, and tensor_tensor_reduce crashes the device).
    """
    if eng is None:
        eng = nc.gpsimd if USE_GPSIMD else nc.vector
    flat = raw.rearrange("p n d -> p (n d)")
    sq = pool.tile([128, ntiles * D], F32, tag=f"{tag}_sq")
    n2 = pool.tile([128, ntiles], F32, tag=f"{tag}_n2")
    eng.tensor_mul(sq, flat, flat)
    nc.vector.tensor_reduce(
        n2, sq.rearrange("p (n d) -> p n d", d=D), axis=mybir.AxisListType.X,
        op=ALU.add)
    inv = _emit_rsqrt(nc, pool, n2, ntiles, eng=eng)
    pn = pool.tile([128, ntiles, D], F32, tag=f"{tag}_pn")
    for n in range(ntiles):
        eng.tensor_scalar_mul(pn[:, n, :], raw[:, n, :],
                              inv[:, n:n + 1])
    return pn


def _emit_normalize_act(nc, pool, raw, ntiles, tag, out_dtype=None):
    """Prologue-only variant: squares and scale-muls run on the scalar
    engine (idle before the exp stream starts; Square/Copy live in every ACT
    table set so no swap), reduce+rsqrt on DVE.  Must not be used once the
    exp stream is running -- ACT queue order would stall it."""
    flat = raw.rearrange("p n d -> p (n d)")
    sq = pool.tile([128, ntiles * D], F32, tag=f"{tag}_sq")
    n2 = pool.tile([128, ntiles], F32, tag=f"{tag}_n2")
    nc.scalar.square(sq, flat)
    nc.vector.tensor_reduce(
        n2, sq.rearrange("p (n d) -> p n d", d=D), axis=mybir.AxisListType.X,
        op=ALU.add)
    inv = _emit_rsqrt(nc, pool, n2, ntiles)
    dt = out_dtype or F32
    pn = pool.tile([128, ntiles, D], dt, tag=f"{tag}_pn")
    for n in range(ntiles):
        nc.scalar.mul(pn[:, n, :], raw[:, n, :], inv[:, n:n + 1])
    return pn


def build_program():
    nc = bacc.Bacc("TRN2", target_bir_lowering=False, debug=False,
                   num_devices=NCORES)

    p_d = nc.dram_tensor("p", [M, D], F32, kind="ExternalInput").ap()
    psq_d = nc.dram_tensor("psq", [2 * R, D], F32,
                           kind="ExternalInput").ap()
    probs_d = nc.dram_tensor("probs", [R, NCLS], F32, kind="ExternalInput").ap()
    iota_d = nc.dram_tensor("iotah", [128, NCLS], F32,
                            kind="ExternalInput").ap()
    tgtr_d = nc.dram_tensor("tgtrep", [128, NS, NCLS], F32,
                            kind="ExternalInput").ap()
    out_d = nc.dram_tensor("out", [1, 2], F32, kind="ExternalOutput").ap()

    MMDT = BF16 if USE_BF16_MM else F32
    G = 4                 # stage-A column groups of the full p
    NTG = NT // G         # 16 row-chunks per group
    JJ = 8                # col groups of 1024 in the main loop
    # Bresenham split of the 64 row-sum reductions between ACT accum_out and
    # DVE tensor_reduce, to balance the two engines.
    N_ACC = N_ACC_ENV

    with tile.TileContext(nc) as tc:
        import contextlib
        with contextlib.ExitStack() as ctx:
            if BENCH_REPS > 1:
                ctx.enter_context(tc.For_i(0, BENCH_REPS, 1))
            consts = ctx.enter_context(tc.tile_pool(name="consts", bufs=1))
            big = ctx.enter_context(tc.tile_pool(name="big", bufs=1))
            work = ctx.enter_context(tc.tile_pool(name="work", bufs=2))
            grp = ctx.enter_context(tc.tile_pool(name="grp", bufs=2))
            tp = ctx.enter_context(
                tc.tile_pool(name="tp", bufs=3, space="PSUM"))
            mm = ctx.enter_context(
                tc.tile_pool(name="mm", bufs=2, space="PSUM"))
            po = ctx.enter_context(
                tc.tile_pool(name="po", bufs=1, space="PSUM"))
            esc = ctx.enter_context(tc.tile_pool(name="esc", bufs=6))

            identity = consts.tile([128, 128], MMDT)
            make_identity(nc, identity)
            iota10 = consts.tile([128, NCLS], F32)
            nc.sync.dma_start(out=iota10, in_=iota_d)
            ones = consts.tile([128, 1], F32)
            nc.vector.memset(ones, 1.0)

            eng = nc.gpsimd if USE_GPSIMD else nc.vector
            GROUPS = [16, 16, 16, 16]         # chunks per group, sum = NT
            goff = [0, 16, 32, 48]
            pnT = big.tile([64, M], MMDT)

            # prologue DMAs first so both chains can start immediately
            rawsq = big.tile([128, 2 * NS, D], F32)
            psq_r = psq_d.rearrange("(n p) d -> p n d", p=128)
            nc.sync.dma_start(out=rawsq[:, 0:NS, :], in_=psq_r[:, 0:NS, :])
            g0_raw = grp.tile([128, GROUPS[0], D], F32, tag="rawg",
                              padded_shape=[128, max(GROUPS), D])
            nc.sync.dma_start(
                out=g0_raw,
                in_=p_d.rearrange("(n p) d -> p n d", p=128)[
                    :, 0:GROUPS[0], :])
            nc.sync.dma_start(out=rawsq[:, NS:2 * NS, :],
                              in_=psq_r[:, NS:2 * NS, :])

            # Fine-grained prologue: compute the two rsqrt chains up front,
            # then interleave per-4-chunk normalize+transpose+copy so the
            # first psT / pnT columns (and with them the exp stream) are
            # ready as early as possible.  All on DVE: any op queued on ACT
            # ahead of the exps would head-of-line-block the stream.
            # ps half only (critical path to psT); pp half is tail-only
            # and is normalized after the stream has started.
            sflat = rawsq[:, 0:NS, :].rearrange("p n d -> p (n d)")
            s_sq = big.tile([128, NS * D], F32)
            s_n2 = big.tile([128, NS], F32)
            nc.vector.tensor_mul(s_sq, sflat, sflat)
            nc.vector.tensor_reduce(
                s_n2, s_sq.rearrange("p (n d) -> p n d", d=D),
                axis=mybir.AxisListType.X, op=ALU.add)
            s_inv = _emit_rsqrt(nc, big, s_n2, NS)

            gflat = g0_raw.rearrange("p n d -> p (n d)")
            g0_sq = grp.tile([128, GROUPS[0] * D], F32, tag="sqg",
                             padded_shape=[128, max(GROUPS) * D])
            g0_n2 = grp.tile([128, GROUPS[0]], F32, tag="n2g",
                             padded_shape=[128, max(GROUPS)])
            nc.vector.tensor_mul(g0_sq, gflat, gflat)
            nc.vector.tensor_reduce(
                g0_n2, g0_sq.rearrange("p (n d) -> p n d", d=D),
                axis=mybir.AxisListType.X, op=ALU.add)
            g0_inv = _emit_rsqrt(nc, grp, g0_n2, GROUPS[0], eng=eng)

            pnsq = big.tile([128, 2 * NS, D], F32)
            pns = pnsq[:, 0:NS, :]
            pnp = pnsq[:, NS:2 * NS, :]
            pnsb = pnsq if not USE_BF16_MM else big.tile([128, NS, D], BF16)
            psT = big.tile([64, R], MMDT)
            g0_pn = grp.tile([128, GROUPS[0], D], MMDT, tag="png",
                             padded_shape=[128, max(GROUPS), D])

            # shard chunks 0-3 -> psT[:, 0:512] first, then g0 chunks 0-7
            # -> pnT[:, 0:1024], then the remaining chunks of each.
            def shard_quad(q4):
                for n in range(4 * q4, 4 * q4 + 4):
                    nc.vector.tensor_scalar_mul(pnsq[:, n, :],
                                                rawsq[:, n, :],
                                                s_inv[:, n:n + 1])
                if USE_BF16_MM:
                    nc.vector.tensor_copy(
                        pnsb[:, 4 * q4:4 * q4 + 4, :].rearrange(
                            "p n d -> p (n d)"),
                        pnsq[:, 4 * q4:4 * q4 + 4, :].rearrange(
                            "p n d -> p (n d)"))
                tpp = tp.tile([64, 512], MMDT, tag="tp")
                for q in range(4):
                    nn = 4 * q4 + q
                    nc.tensor.transpose(
                        tpp[:, q * 128:(q + 1) * 128], pnsb[:, nn, :],
                        identity)
                nc.vector.tensor_copy(psT[:, q4 * 512:(q4 + 1) * 512], tpp)

            def g0_quad(q4):
                for n in range(4 * q4, 4 * q4 + 4):
                    nc.vector.tensor_scalar_mul(g0_pn[:, n, :],
                                                g0_raw[:, n, :],
                                                g0_inv[:, n:n + 1])
                tpp = tp.tile([64, 512], MMDT, tag="tp")
                for q in range(4):
                    nn = 4 * q4 + q
                    nc.tensor.transpose(
                        tpp[:, q * 128:(q + 1) * 128], g0_pn[:, nn, :],
                        identity)
                nc.vector.tensor_copy(pnT[:, q4 * 512:(q4 + 1) * 512], tpp)

            shard_quad(0)
            g0_quad(0)
            g0_quad(1)
            shard_quad(1)
            g0_quad(2)
            g0_quad(3)

            # pp half chain (tail-only data, after the stream is rolling)
            pflat = rawsq[:, NS:2 * NS, :].rearrange("p n d -> p (n d)")
            p_sq = big.tile([128, NS * D], F32)
            p_n2 = big.tile([128, NS], F32)
            nc.vector.tensor_mul(p_sq, pflat, pflat)
            nc.vector.tensor_reduce(
                p_n2, p_sq.rearrange("p (n d) -> p n d", d=D),
                axis=mybir.AxisListType.X, op=ALU.add)
            p_inv = _emit_rsqrt(nc, big, p_n2, NS)
            for n in range(NS):
                nc.vector.tensor_scalar_mul(pnsq[:, NS + n, :],
                                            rawsq[:, NS + n, :],
                                            p_inv[:, n:n + 1])

            probs_t = big.tile([128, NS, NCLS], F32)
            nc.sync.dma_start(
                out=probs_t, in_=probs_d.rearrange("(n p) c -> p n c", p=128))
            tgtr_t = big.tile([128, NS, NCLS], F32)
            nc.sync.dma_start(out=tgtr_t, in_=tgtr_d)
            eprobs = big.tile([128, NS, NCLS], F32)
            nc.scalar.activation(
                eprobs.rearrange("p n c -> p (n c)"),
                probs_t.rearrange("p n c -> p (n c)"), AF.Exp)

            # ---- full p, remaining pipelined column groups ----
            scols = big.tile([128, NS * JJ], F32)
            for g, ntg in enumerate(GROUPS):
                if g == 0:
                    png = None  # prologue already produced pnT cols
                else:
                    rawg = grp.tile([128, ntg, D], F32, tag="rawg",
                                    padded_shape=[128, max(GROUPS), D])
                    nc.sync.dma_start(
                        out=rawg,
                        in_=p_d.rearrange("(n p) d -> p n d", p=128)[
                            :, goff[g]:goff[g] + ntg, :])
                    flat = rawg.rearrange("p n d -> p (n d)")
                    sqg = grp.tile([128, ntg * D], F32, tag="sqg",
                                   padded_shape=[128, max(GROUPS) * D])
                    n2g = grp.tile([128, ntg], F32, tag="n2g",
                                   padded_shape=[128, max(GROUPS)])
                    eng.tensor_mul(sqg, flat, flat)
                    nc.vector.tensor_reduce(
                        n2g, sqg.rearrange("p (n d) -> p n d", d=D),
                        axis=mybir.AxisListType.X, op=ALU.add)
                    invg = _emit_rsqrt(nc, grp, n2g, ntg,
                                       eng=eng)
                    png = grp.tile([128, ntg, D], MMDT, tag="png",
                                   padded_shape=[128, max(GROUPS), D])
                    for n in range(ntg):
                        eng.tensor_scalar_mul(png[:, n, :], rawg[:, n, :],
                                              invg[:, n:n + 1])
                if g != 0:
                    for t4 in range(ntg // 4):
                        tpp = tp.tile([64, 512], MMDT, tag="tp")
                        for q in range(4):
                            nn = 4 * t4 + q
                            nc.tensor.transpose(
                                tpp[:, q * 128:(q + 1) * 128], png[:, nn, :],
                                identity)
                        col = (goff[g] + t4 * 4) * 128
                        nc.vector.tensor_copy(pnT[:, col:col + 512], tpp)

                # main loop for this group's columns
                jlo = goff[g] * 128 // 1024
                jhi = (goff[g] + ntg) * 128 // 1024
                for jj in range(jlo, jhi):
                    c0 = jj * 1024
                    for n in range(NS):
                        idx = n * JJ + jj
                        pst = mm.tile([128, 1024], F32, tag="mm")
                        lhsT = psT[:, n * 128:(n + 1) * 128]
                        nc.tensor.matmul(pst[:, 0:512], lhsT,
                                         pnT[:, c0:c0 + 512],
                                         start=True, stop=True)
                        nc.tensor.matmul(pst[:, 512:1024], lhsT,
                                         pnT[:, c0 + 512:c0 + 1024],
                                         start=True, stop=True)
                        et = esc.tile([128, 1024], F32, tag="esc")
                        if (idx * N_ACC) % (NS * JJ) < N_ACC:
                            nc.scalar.activation(
                                et, pst, AF.Exp, scale=INV_TEMP,
                                accum_out=scols[:, idx:idx + 1])
                        else:
                            nc.scalar.activation(et, pst, AF.Exp,
                                                 scale=INV_TEMP)
                            nc.vector.tensor_reduce(
                                scols[:, idx:idx + 1], et,
                                axis=mybir.AxisListType.X, op=ALU.add)

            sum10 = big.tile([128, NS], F32)
            nc.vector.tensor_reduce(sum10, eprobs, axis=mybir.AxisListType.X,
                                    op=ALU.add)
            own = big.tile([128, NS], F32)
            for n in range(NS):
                mask = work.tile([128, NCLS], F32, tag="mask")
                nc.vector.tensor_tensor(mask, iota10, tgtr_t[:, n, :],
                                        ALU.is_equal)
                nc.vector.tensor_mul(mask, mask, eprobs[:, n, :])
                nc.vector.tensor_reduce(own[:, n:n + 1], mask,
                                        axis=mybir.AxisListType.X, op=ALU.add)

            # pos_r and diag_r row-dots in fp32 (raw, without *2 temp
            # scale) — emitted late so their DVE/ACT ops cannot stall the
            # main exp stream (engine queues respect program order).
            diag_raw = big.tile([128, NS], F32)
            pos_raw = big.tile([128, NS], F32)
            dq = work.tile([128, NS, D], F32, tag="rowdot", bufs=2)
            nc.vector.tensor_mul(dq, pns, pns)
            nc.vector.tensor_reduce(diag_raw, dq, axis=mybir.AxisListType.X,
                                    op=ALU.add)
            pq = work.tile([128, NS, D], F32, tag="rowdot", bufs=2)
            nc.vector.tensor_mul(pq, pns, pnp)
            nc.vector.tensor_reduce(pos_raw, pq, axis=mybir.AxisListType.X,
                                    op=ALU.add)
            ediag = big.tile([128, NS], F32)
            nc.scalar.activation(ediag, diag_raw, AF.Exp, scale=INV_TEMP)
            epos = big.tile([128, NS], F32)
            nc.scalar.activation(epos, pos_raw, AF.Exp, scale=INV_TEMP)
            pos2 = big.tile([128, NS], F32)
            nc.vector.tensor_scalar_mul(pos2, pos_raw, INV_TEMP)

            # ---- loss tails ----
            stot = big.tile([128, NS], F32)
            nc.vector.tensor_reduce(
                stot, scols.rearrange("p (n j) -> p n j", j=JJ),
                axis=mybir.AxisListType.X, op=ALU.add)
            s1 = big.tile([128, NS], F32)
            nc.vector.tensor_sub(s1, stot, ediag)
            lse1 = big.tile([128, NS], F32)
            nc.scalar.activation(lse1, s1, AF.Ln)
            c1 = big.tile([128, NS], F32)
            nc.vector.tensor_sub(c1, lse1, pos2)
            v12 = big.tile([128, 2], F32)
            nc.vector.tensor_reduce(v12[:, 0:1], c1,
                                    axis=mybir.AxisListType.X, op=ALU.add)

            s2 = big.tile([128, NS], F32)
            nc.vector.tensor_sub(s2, sum10, own)
            nc.vector.tensor_add(s2, s2, epos)
            # false data-dep on stot so the scheduler cannot hoist the Ln
            # into the exp stream (each hoist costs 2 ACT table swaps)
            nc.vector.scalar_tensor_tensor(
                out=s2, in0=stot, scalar=0.0, in1=s2,
                op0=ALU.mult, op1=ALU.add)
            lse2 = big.tile([128, NS], F32)
            nc.scalar.activation(lse2, s2, AF.Ln)
            c2 = big.tile([128, NS], F32)
            nc.vector.tensor_sub(c2, lse2, pos2)
            nc.vector.tensor_reduce(v12[:, 1:2], c2,
                                    axis=mybir.AxisListType.X, op=ALU.add)

            # ---- partition-sum via ones-matmul, then DMA out ----
            pso = po.tile([1, 2], F32)
            nc.tensor.matmul(pso, ones, v12, start=True, stop=True)
            outsb = big.tile([1, 2], F32)
            nc.vector.tensor_copy(outsb, pso)
            nc.sync.dma_start(out=out_d, in_=outsb)

    nc.compile()
    return nc


_NC_CACHE = None


def _get_nc():
    global _NC_CACHE
    if _NC_CACHE is None:
        _NC_CACHE = build_program()
    return _NC_CACHE


def make_in_maps(z_i, z_j, probs, target):
    p = np.ascontiguousarray(
        np.concatenate([z_i, z_j], axis=0), dtype=np.float32)
    t2 = np.concatenate([target, target]).astype(np.float32)
    probs = np.asarray(probs, dtype=np.float32)
    iotah = np.broadcast_to(np.arange(NCLS, dtype=np.float32),
                            (128, NCLS)).copy()
    in_maps = []
    for k in range(NCORES):
        lo = k * R
        plo = (lo + N) % M
        # tgtrep[p, n, c] = t2[lo + n*128 + p] for all c
        tgt_k = t2[lo:lo + R].reshape(NS, 128).T          # [128, NS]
        tgtrep = np.ascontiguousarray(
            np.repeat(tgt_k[:, :, None], NCLS, axis=2), dtype=np.float32)
        in_maps.append({
            "p": p,
            "psq": np.ascontiguousarray(
                np.concatenate([p[lo:lo + R], p[plo:plo + R]], axis=0)),
            "probs": np.ascontiguousarray(probs[lo:lo + R]),
            "iotah": iotah,
            "tgtrep": tgtrep,
        })
    return in_maps


def kernel(z_i, z_j, probs, target, neg_idx):
    # neg_idx is the fixed structured NT-Xent mask (all columns except self and
    # positive); its effect is computed analytically, so it is never read.
    del neg_idx
    nc = _get_nc()
    in_maps = make_in_maps(np.asarray(z_i), np.asarray(z_j),
                           np.asarray(probs), np.asarray(target))
    res = run_bass_kernel_spmd(nc, in_maps, list(range(NCORES)))
    parts = np.stack([res.results[k]["out"].reshape(2) for k in range(NCORES)])
    total = parts.sum(axis=0) / np.float32(M)
    l1 = np.float32(total[0])
    l2 = np.float32(total[1])
    return (np.asarray(l1), np.asarray(l2))



# revision 2
# speedup vs baseline: 3.1329x; 3.1329x over previous
"""NT-Xent loss kernel for Trainium2, 8-core SPMD.

Math: with p = cat(z_i, z_j) [8192, 64], pn = p / max(||p||, 1e-8),
sim = 2 * pn @ pn.T (TEMP=0.5), the reference's gather-based losses reduce to
  loss1 = mean_r( log(sum_{c != r} exp(sim[r,c])) - pos_r )
  loss2 = mean_r( log(exp(pos_r) + sum_{c != t_r} exp(probs[r,c])) - pos_r )
where pos_r = sim[r, (r+N) % 2N].  sim entries lie in [-2, 2], so the exp
never overflows and no max-shift pass is needed.  The huge neg_idx input is a
fixed structured mask (drop self + positive) and never needs to be read.

Sharding: row-parallel.  Each of the 8 cores receives ONLY its own 1024 rows
of p (256 KB) plus its probs/target slices; it normalizes them locally, then
an on-device AllGather of the normalized rows builds the full [8192, 64] pn
every core needs for the sim columns.  The positive-pair rows pn[(r+N)%2N]
live on core k^4; a second, pairwise AllGather over groups {k, k+4} brings
them in, and pos is recovered order-independently as
  pos = pns . (lo + hi) - diag        (lo+hi = own + partner rows)
so the same SPMD program works on every rank with static addressing.
Each core emits two partial sums; the host adds the 8 partials.

Wall-clock notes (the metric here is warm dispatch wall time; the axon
NTFF-profile path is unavailable in this container):
  - inputs shrank 21.7 MB -> 2.4 MB per call (the dominant cost was host->
    device transfer over the axon tunnel),
  - the jax persistent compilation cache makes repeat dispatches skip the
    client-side walrus NEFF recompile (~140 ms/call),
  - iota is a NEFF-embedded Const tensor instead of a runtime input.
"""

import numpy as np

import jax

# Persistent compilation cache: run_bass_kernel_spmd builds a fresh jax.jit
# wrapper per call, so without this every dispatch re-runs the client-side
# walrus NEFF compile. With it, identical HLO (same BIR) is a disk cache hit.
try:
    jax.config.update("jax_compilation_cache_dir", "/tmp/jax_comp_cache_ntx")
    jax.config.update("jax_persistent_cache_min_compile_time_secs", 0)
    jax.config.update("jax_persistent_cache_min_entry_size_bytes", -1)
except Exception:
    pass

import concourse.bass as bass
import concourse.bacc as bacc
import concourse.tile as tile
from concourse import mybir
from concourse.masks import make_identity
from concourse.bass_utils import run_bass_kernel_spmd

N = 4096
D = 64
M = 2 * N            # 8192 rows of sim
NCORES = 8
R = M // NCORES      # 1024 rows per core
NT = M // 128        # 64 row-tiles of the full p
NS = R // 128        # 8 row-tiles of a shard
NCLS = 10
INV_TEMP = 2.0       # 1 / 0.5
F32 = mybir.dt.float32
BF16 = mybir.dt.bfloat16

# bf16 matmul for the sim slab: 4x PE throughput, 2x moving-dim. pos/diag
# stay fp32 (computed on DVE), and per-row errors average out over 8192 rows.
import os
USE_BF16_MM = os.environ.get("NTX_BF16", "1") == "1"
N_ACC_ENV = int(os.environ.get("NTX_NACC", "28"))
NEWTON_ITERS = int(os.environ.get("NTX_NEWT", "2"))

AF = mybir.ActivationFunctionType
ALU = mybir.AluOpType


def _emit_rsqrt(nc, pool, n2, nchunk, eng=None):
    """inv = 1/max(sqrt(n2), 1e-8), entirely on DVE: quake-style magic
    constant seed + Newton steps (ACT stays exclusively on Exp/Ln, so the
    activation table never thrashes)."""
    if eng is None:
        eng = nc.vector
    I32 = mybir.dt.int32
    inv = pool.tile([128, nchunk], F32, tag="rs_inv")
    eng.tensor_scalar(inv.bitcast(I32), n2.bitcast(I32), 1, None,
                      ALU.arith_shift_right)
    eng.tensor_scalar(inv.bitcast(I32), inv.bitcast(I32), -1, 0x5F3759DF,
                      ALU.mult, ALU.add)
    t2 = pool.tile([128, nchunk], F32, tag="rs_t2")
    for _ in range(NEWTON_ITERS):
        # y' = y * (1.5 - 0.5 * n2 * y^2)
        eng.tensor_mul(t2, inv, inv)
        eng.tensor_mul(t2, t2, n2)
        eng.tensor_scalar(t2, t2, -0.5, 1.5, ALU.mult, ALU.add)
        eng.tensor_mul(inv, inv, t2)
    eng.tensor_scalar_min(inv, inv, 1e8)
    return inv


def build_program():
    nc = bacc.Bacc("TRN2", target_bir_lowering=False, debug=False,
                   num_devices=NCORES)

    zsh_d = nc.dram_tensor("zsh", [R, D], F32, kind="ExternalInput").ap()
    probs_d = nc.dram_tensor("probs", [R, NCLS], F32,
                             kind="ExternalInput").ap()
    tgt_d = nc.dram_tensor("tgt", [NS, 128], F32, kind="ExternalInput").ap()
    out_d = nc.dram_tensor("out", [1, 2], F32, kind="ExternalOutput").ap()

    iota_h = nc.inline_tensor(
        np.broadcast_to(np.arange(NCLS, dtype=np.float32),
                        (128, NCLS)).copy(), name="iotah").ap()

    MMDT = BF16 if USE_BF16_MM else F32
    G = 4                 # pnT build groups over the gathered full p
    NTG = NT // G         # 16 row-chunks per group
    JJ = 8                # col groups of 1024 in the main loop
    N_ACC = N_ACC_ENV     # of the 64 row-sums, how many use ACT accum_out

    with tile.TileContext(nc) as tc:
        import contextlib
        with contextlib.ExitStack() as ctx:
            consts = ctx.enter_context(tc.tile_pool(name="consts", bufs=1))
            big = ctx.enter_context(tc.tile_pool(name="big", bufs=1))
            work = ctx.enter_context(tc.tile_pool(name="work", bufs=2))
            grp = ctx.enter_context(tc.tile_pool(name="grp", bufs=2))
            tp = ctx.enter_context(
                tc.tile_pool(name="tp", bufs=3, space="PSUM"))
            mm = ctx.enter_context(
                tc.tile_pool(name="mm", bufs=2, space="PSUM"))
            po = ctx.enter_context(
                tc.tile_pool(name="po", bufs=1, space="PSUM"))
            esc = ctx.enter_context(tc.tile_pool(name="esc", bufs=6))
            dr = ctx.enter_context(
                tc.tile_pool(name="dr", bufs=1, space="DRAM"))

            identity = consts.tile([128, 128], MMDT)
            make_identity(nc, identity)
            iota10 = consts.tile([128, NCLS], F32)
            nc.sync.dma_start(out=iota10, in_=iota_h)
            ones = consts.tile([128, 1], F32)
            nc.vector.memset(ones, 1.0)

            # ---- load + normalize this core's shard (fp32, DVE) ----
            rawsq = big.tile([128, NS, D], F32)
            nc.sync.dma_start(
                out=rawsq, in_=zsh_d.rearrange("(n p) d -> p n d", p=128))

            sflat = rawsq.rearrange("p n d -> p (n d)")
            s_sq = big.tile([128, NS * D], F32)
            s_n2 = big.tile([128, NS], F32)
            nc.vector.tensor_mul(s_sq, sflat, sflat)
            nc.vector.tensor_reduce(
                s_n2, s_sq.rearrange("p (n d) -> p n d", d=D),
                axis=mybir.AxisListType.X, op=ALU.add)
            s_inv = _emit_rsqrt(nc, big, s_n2, NS)
            pns = big.tile([128, NS, D], F32)
            for n in range(NS):
                nc.vector.tensor_scalar_mul(pns[:, n, :], rawsq[:, n, :],
                                            s_inv[:, n:n + 1])

            # ---- on-device gathers of the normalized rows ----
            agin = dr.tile([R, D], F32)
            nc.sync.dma_start(
                out=agin.rearrange("(n p) d -> p n d", p=128), in_=pns)
            agfull = dr.tile([M, D], F32, addr_space="Shared")
            nc.gpsimd.collective_compute(
                "AllGather", ALU.bypass,
                replica_groups=[list(range(NCORES))],
                ins=[agin], outs=[agfull])
            agpair = dr.tile([2 * R, D], F32)
            nc.gpsimd.collective_compute(
                "AllGather", ALU.bypass,
                replica_groups=[[k, k + 4] for k in range(4)],
                ins=[agin], outs=[agpair])

            # bf16 copy of own shard + transposes -> psT [64, R]
            pnsb = pns if not USE_BF16_MM else big.tile([128, NS, D], BF16)
            if USE_BF16_MM:
                nc.vector.tensor_copy(
                    pnsb.rearrange("p n d -> p (n d)"),
                    pns.rearrange("p n d -> p (n d)"))
            psT = big.tile([64, R], MMDT)
            for q4 in range(NS // 4):
                tpp = tp.tile([64, 512], MMDT, tag="tp")
                for q in range(4):
                    nn = 4 * q4 + q
                    nc.tensor.transpose(
                        tpp[:, q * 128:(q + 1) * 128], pnsb[:, nn, :],
                        identity)
                nc.vector.tensor_copy(psT[:, q4 * 512:(q4 + 1) * 512], tpp)

            # probs part (independent of the gathers; ACT stays on Exp)
            probs_t = big.tile([128, NS, NCLS], F32)
            nc.sync.dma_start(
                out=probs_t, in_=probs_d.rearrange("(n p) c -> p n c", p=128))
            tgt_t = big.tile([128, NS], F32)
            nc.sync.dma_start(out=tgt_t, in_=tgt_d.rearrange("n p -> p n"))
            eprobs = big.tile([128, NS, NCLS], F32)
            nc.scalar.activation(
                eprobs.rearrange("p n c -> p (n c)"),
                probs_t.rearrange("p n c -> p (n c)"), AF.Exp)

            # ---- full pn columns: convert + transpose per group, with the
            # exp stream of earlier groups overlapping later group builds ----
            pnT = big.tile([64, M], MMDT)
            agv = agfull.rearrange("(n p) d -> p n d", p=128)
            scols = big.tile([128, NS * JJ], F32)
            for g in range(G):
                rawg = grp.tile([128, NTG, D], F32, tag="rawg")
                nc.sync.dma_start(
                    out=rawg, in_=agv[:, g * NTG:(g + 1) * NTG, :])
                if USE_BF16_MM:
                    png = grp.tile([128, NTG, D], BF16, tag="png")
                    nc.vector.tensor_copy(
                        png.rearrange("p n d -> p (n d)"),
                        rawg.rearrange("p n d -> p (n d)"))
                else:
                    png = rawg
                for t4 in range(NTG // 4):
                    tpp = tp.tile([64, 512], MMDT, tag="tp")
                    for q in range(4):
                        nn = 4 * t4 + q
                        nc.tensor.transpose(
                            tpp[:, q * 128:(q + 1) * 128], png[:, nn, :],
                            identity)
                    col = (g * NTG + t4 * 4) * 128
                    nc.vector.tensor_copy(pnT[:, col:col + 512], tpp)

                # main loop over this group's 1024-wide column blocks
                jlo = g * NTG * 128 // 1024
                jhi = (g + 1) * NTG * 128 // 1024
                for jj in range(jlo, jhi):
                    c0 = jj * 1024
                    for n in range(NS):
                        idx = n * JJ + jj
                        pst = mm.tile([128, 1024], F32, tag="mm")
                        lhsT = psT[:, n * 128:(n + 1) * 128]
                        nc.tensor.matmul(pst[:, 0:512], lhsT,
                                         pnT[:, c0:c0 + 512],
                                         start=True, stop=True)
                        nc.tensor.matmul(pst[:, 512:1024], lhsT,
                                         pnT[:, c0 + 512:c0 + 1024],
                                         start=True, stop=True)
                        et = esc.tile([128, 1024], F32, tag="esc")
                        if (idx * N_ACC) % (NS * JJ) < N_ACC:
                            nc.scalar.activation(
                                et, pst, AF.Exp, scale=INV_TEMP,
                                accum_out=scols[:, idx:idx + 1])
                        else:
                            nc.scalar.activation(et, pst, AF.Exp,
                                                 scale=INV_TEMP)
                            nc.vector.tensor_reduce(
                                scols[:, idx:idx + 1], et,
                                axis=mybir.AxisListType.X, op=ALU.add)

            sum10 = big.tile([128, NS], F32)
            nc.vector.tensor_reduce(sum10, eprobs, axis=mybir.AxisListType.X,
                                    op=ALU.add)
            own = big.tile([128, NS], F32)
            for n in range(NS):
                mask = work.tile([128, NCLS], F32, tag="mask")
                nc.vector.tensor_scalar(mask, iota10, tgt_t[:, n:n + 1], None,
                                        ALU.is_equal)
                nc.vector.tensor_mul(mask, mask, eprobs[:, n, :])
                nc.vector.tensor_reduce(own[:, n:n + 1], mask,
                                        axis=mybir.AxisListType.X, op=ALU.add)

            # pos_r and diag_r in fp32 — pair-gathered rows give
            # lo+hi = own + partner, so pos = pns.(lo+hi) - diag with static
            # addressing on every rank.  Emitted late so their DVE/ACT ops
            # cannot stall the main exp stream.
            prl = big.tile([128, NS, D], F32)
            nc.sync.dma_start(
                out=prl,
                in_=agpair[0:R, :].rearrange("(n p) d -> p n d", p=128))
            prh = big.tile([128, NS, D], F32)
            nc.sync.dma_start(
                out=prh,
                in_=agpair[R:2 * R, :].rearrange("(n p) d -> p n d", p=128))
            psum_rows = work.tile([128, NS, D], F32, tag="rowdot", bufs=2)
            nc.vector.tensor_add(psum_rows, prl, prh)

            diag_raw = big.tile([128, NS], F32)
            dsum_raw = big.tile([128, NS], F32)
            dq = work.tile([128, NS, D], F32, tag="rowdot", bufs=2)
            nc.vector.tensor_mul(dq, pns, pns)
            nc.vector.tensor_reduce(diag_raw, dq, axis=mybir.AxisListType.X,
                                    op=ALU.add)
            pq = work.tile([128, NS, D], F32, tag="rowdot", bufs=2)
            nc.vector.tensor_mul(pq, pns, psum_rows)
            nc.vector.tensor_reduce(dsum_raw, pq, axis=mybir.AxisListType.X,
                                    op=ALU.add)
            pos_raw = big.tile([128, NS], F32)
            nc.vector.tensor_sub(pos_raw, dsum_raw, diag_raw)

            ediag = big.tile([128, NS], F32)
            nc.scalar.activation(ediag, diag_raw, AF.Exp, scale=INV_TEMP)
            epos = big.tile([128, NS], F32)
            nc.scalar.activation(epos, pos_raw, AF.Exp, scale=INV_TEMP)
            pos2 = big.tile([128, NS], F32)
            nc.vector.tensor_scalar_mul(pos2, pos_raw, INV_TEMP)

            # ---- loss tails ----
            stot = big.tile([128, NS], F32)
            nc.vector.tensor_reduce(
                stot, scols.rearrange("p (n j) -> p n j", j=JJ),
                axis=mybir.AxisListType.X, op=ALU.add)
            s1 = big.tile([128, NS], F32)
            nc.vector.tensor_sub(s1, stot, ediag)
            lse1 = big.tile([128, NS], F32)
            nc.scalar.activation(lse1, s1, AF.Ln)
            c1 = big.tile([128, NS], F32)
            nc.vector.tensor_sub(c1, lse1, pos2)
            v12 = big.tile([128, 2], F32)
            nc.vector.tensor_reduce(v12[:, 0:1], c1,
                                    axis=mybir.AxisListType.X, op=ALU.add)

            s2 = big.tile([128, NS], F32)
            nc.vector.tensor_sub(s2, sum10, own)
            nc.vector.tensor_add(s2, s2, epos)
            # false data-dep on stot so the scheduler cannot hoist the Ln
            # into the exp stream (each hoist costs 2 ACT table swaps)
            nc.vector.scalar_tensor_tensor(
                out=s2, in0=stot, scalar=0.0, in1=s2,
                op0=ALU.mult, op1=ALU.add)
            lse2 = big.tile([128, NS], F32)
            nc.scalar.activation(lse2, s2, AF.Ln)
            c2 = big.tile([128, NS], F32)
            nc.vector.tensor_sub(c2, lse2, pos2)
            nc.vector.tensor_reduce(v12[:, 1:2], c2,
                                    axis=mybir.AxisListType.X, op=ALU.add)

            # ---- partition-sum via ones-matmul, then DMA out ----
            pso = po.tile([1, 2], F32)
            nc.tensor.matmul(pso, ones, v12, start=True, stop=True)
            outsb = big.tile([1, 2], F32)
            nc.vector.tensor_copy(outsb, pso)
            nc.sync.dma_start(out=out_d, in_=outsb)

    nc.compile()
    return nc


_NC_CACHE = None


def _get_nc():
    global _NC_CACHE
    if _NC_CACHE is None:
        _NC_CACHE = build_program()
    return _NC_CACHE


def make_in_maps(z_i, z_j, probs, target):
    p = np.ascontiguousarray(
        np.concatenate([z_i, z_j], axis=0), dtype=np.float32)
    t2 = np.concatenate([target, target]).astype(np.float32)
    probs = np.asarray(probs, dtype=np.float32)
    in_maps = []
    for k in range(NCORES):
        lo = k * R
        in_maps.append({
            "zsh": np.ascontiguousarray(p[lo:lo + R]),
            "probs": np.ascontiguousarray(probs[lo:lo + R]),
            "tgt": np.ascontiguousarray(t2[lo:lo + R].reshape(NS, 128)),
        })
    return in_maps


def kernel(z_i, z_j, probs, target, neg_idx):
    # neg_idx is the fixed structured NT-Xent mask (all columns except self and
    # positive); its effect is computed analytically, so it is never read.
    del neg_idx
    nc = _get_nc()
    in_maps = make_in_maps(np.asarray(z_i), np.asarray(z_j),
                           np.asarray(probs), np.asarray(target))
    res = run_bass_kernel_spmd(nc, in_maps, list(range(NCORES)))
    parts = np.stack([res.results[k]["out"].reshape(2) for k in range(NCORES)])
    total = parts.sum(axis=0) / np.float32(M)
    l1 = np.float32(total[0])
    l2 = np.float32(total[1])
    return (np.asarray(l1), np.asarray(l2))


# revision 6
# speedup vs baseline: 3.8563x; 1.2309x over previous
"""NT-Xent loss kernel for Trainium2, 8-core SPMD.

Math: with p = cat(z_i, z_j) [8192, 64], pn = p / max(||p||, 1e-8),
sim = 2 * pn @ pn.T (TEMP=0.5), the reference's gather-based losses reduce to
  loss1 = mean_r( log(sum_{c != r} exp(sim[r,c])) - pos_r )
  loss2 = mean_r( log(exp(pos_r) + sum_{c != t_r} exp(probs[r,c])) - pos_r )
where pos_r = sim[r, (r+N) % 2N].  sim entries lie in [-2, 2], so the exp
never overflows and no max-shift pass is needed.  The huge neg_idx input is a
fixed structured mask (drop self + positive) and never needs to be read.

Sharding: row-parallel.  Each of the 8 cores receives ONLY its own 1024 rows
of p (256 KB) plus its probs/target slices; it normalizes them locally, then
an on-device AllGather of the normalized rows builds the full [8192, 64] pn
every core needs for the sim columns.  The positive-pair rows pn[(r+N)%2N]
live on core k^4; a second, pairwise AllGather over groups {k, k+4} brings
them in, and pos is recovered order-independently as
  pos = pns . (lo + hi) - diag        (lo+hi = own + partner rows)
so the same SPMD program works on every rank with static addressing.
Each core emits two partial sums; the host adds the 8 partials.

Wall-clock notes (the metric here is warm dispatch wall time; the axon
NTFF-profile path is unavailable in this container):
  - inputs shrank 21.7 MB -> 2.4 MB per call (the dominant cost was host->
    device transfer over the axon tunnel),
  - the jax persistent compilation cache makes repeat dispatches skip the
    client-side walrus NEFF recompile (~140 ms/call),
  - iota is a NEFF-embedded Const tensor instead of a runtime input.
"""

import numpy as np

import jax

# Persistent compilation cache: run_bass_kernel_spmd builds a fresh jax.jit
# wrapper per call, so without this every dispatch re-runs the client-side
# walrus NEFF compile. With it, identical HLO (same BIR) is a disk cache hit.
try:
    jax.config.update("jax_compilation_cache_dir", "/tmp/jax_comp_cache_ntx")
    jax.config.update("jax_persistent_cache_min_compile_time_secs", 0)
    jax.config.update("jax_persistent_cache_min_entry_size_bytes", -1)
except Exception:
    pass

import concourse.bass as bass
import concourse.bacc as bacc
import concourse.tile as tile
from concourse import mybir
from concourse.masks import make_identity
from concourse.bass_utils import run_bass_kernel_spmd

N = 4096
D = 64
M = 2 * N            # 8192 rows of sim
NCORES = 8
R = M // NCORES      # 1024 rows per core
NT = M // 128        # 64 row-tiles of the full p
NS = R // 128        # 8 row-tiles of a shard
NCLS = 10
INV_TEMP = 2.0       # 1 / 0.5
F32 = mybir.dt.float32
BF16 = mybir.dt.bfloat16

# bf16 matmul for the sim slab: 4x PE throughput, 2x moving-dim. pos/diag
# stay fp32 (computed on DVE), and per-row errors average out over 8192 rows.
import os
USE_BF16_MM = os.environ.get("NTX_BF16", "1") == "1"
N_ACC_ENV = int(os.environ.get("NTX_NACC", "28"))
NEWTON_ITERS = int(os.environ.get("NTX_NEWT", "2"))

AF = mybir.ActivationFunctionType
ALU = mybir.AluOpType


def _emit_rsqrt(nc, pool, n2, nchunk, eng=None):
    """inv = 1/max(sqrt(n2), 1e-8), entirely on DVE: quake-style magic
    constant seed + Newton steps (ACT stays exclusively on Exp/Ln, so the
    activation table never thrashes)."""
    if eng is None:
        eng = nc.vector
    I32 = mybir.dt.int32
    inv = pool.tile([128, nchunk], F32, tag="rs_inv")
    eng.tensor_scalar(inv.bitcast(I32), n2.bitcast(I32), 1, None,
                      ALU.arith_shift_right)
    eng.tensor_scalar(inv.bitcast(I32), inv.bitcast(I32), -1, 0x5F3759DF,
                      ALU.mult, ALU.add)
    t2 = pool.tile([128, nchunk], F32, tag="rs_t2")
    for _ in range(NEWTON_ITERS):
        # y' = y * (1.5 - 0.5 * n2 * y^2)
        eng.tensor_mul(t2, inv, inv)
        eng.tensor_mul(t2, t2, n2)
        eng.tensor_scalar(t2, t2, -0.5, 1.5, ALU.mult, ALU.add)
        eng.tensor_mul(inv, inv, t2)
    eng.tensor_scalar_min(inv, inv, 1e8)
    return inv


def build_program():
    nc = bacc.Bacc("TRN2", target_bir_lowering=False, debug=False,
                   num_devices=NCORES)

    # One packed input per core: fewer h2d buffers per dispatch.
    # Layout (f32 elements): [0, R*D) zsh | [R*D, R*D+R*NCLS) probs |
    # [R*D+R*NCLS, R*D+R*NCLS+R) tgt.
    BLOB = R * D + R * NCLS + R
    blob_d = nc.dram_tensor("blob", [1, BLOB], F32,
                            kind="ExternalInput").ap()
    zsh_d = blob_d[0, 0:R * D].rearrange("(n p d) -> p n d", p=128, d=D)
    probs_d = blob_d[0, R * D:R * D + R * NCLS].rearrange(
        "(n p c) -> p n c", p=128, c=NCLS)
    tgt_d = blob_d[0, R * D + R * NCLS:BLOB].rearrange("(n p) -> p n", p=128)
    out_d = nc.dram_tensor("out", [1, 2], F32, kind="ExternalOutput").ap()

    iota_h = nc.inline_tensor(
        np.broadcast_to(np.arange(NCLS, dtype=np.float32),
                        (128, NCLS)).copy(), name="iotah").ap()

    MMDT = BF16 if USE_BF16_MM else F32
    G = 4                 # pnT build groups over the gathered full p
    NTG = NT // G         # 16 row-chunks per group
    JJ = 8                # col groups of 1024 in the main loop
    N_ACC = N_ACC_ENV     # of the 64 row-sums, how many use ACT accum_out

    with tile.TileContext(nc) as tc:
        import contextlib
        with contextlib.ExitStack() as ctx:
            consts = ctx.enter_context(tc.tile_pool(name="consts", bufs=1))
            big = ctx.enter_context(tc.tile_pool(name="big", bufs=1))
            work = ctx.enter_context(tc.tile_pool(name="work", bufs=2))
            grp = ctx.enter_context(tc.tile_pool(name="grp", bufs=2))
            tp = ctx.enter_context(
                tc.tile_pool(name="tp", bufs=3, space="PSUM"))
            mm = ctx.enter_context(
                tc.tile_pool(name="mm", bufs=2, space="PSUM"))
            po = ctx.enter_context(
                tc.tile_pool(name="po", bufs=1, space="PSUM"))
            esc = ctx.enter_context(tc.tile_pool(name="esc", bufs=6))
            dr = ctx.enter_context(
                tc.tile_pool(name="dr", bufs=1, space="DRAM"))

            identity = consts.tile([128, 128], MMDT)
            make_identity(nc, identity)
            iota10 = consts.tile([128, NCLS], F32)
            nc.sync.dma_start(out=iota10, in_=iota_h)
            ones = consts.tile([128, 1], F32)
            nc.vector.memset(ones, 1.0)

            # ---- load + normalize this core's shard (fp32, DVE) ----
            rawsq = big.tile([128, NS, D], F32)
            nc.sync.dma_start(out=rawsq, in_=zsh_d)

            sflat = rawsq.rearrange("p n d -> p (n d)")
            s_sq = big.tile([128, NS * D], F32)
            s_n2 = big.tile([128, NS], F32)
            nc.vector.tensor_mul(s_sq, sflat, sflat)
            nc.vector.tensor_reduce(
                s_n2, s_sq.rearrange("p (n d) -> p n d", d=D),
                axis=mybir.AxisListType.X, op=ALU.add)
            s_inv = _emit_rsqrt(nc, big, s_n2, NS)
            pns = big.tile([128, NS, D], F32)
            for n in range(NS):
                nc.vector.tensor_scalar_mul(pns[:, n, :], rawsq[:, n, :],
                                            s_inv[:, n:n + 1])

            # ---- on-device gathers of the normalized rows ----
            agin = dr.tile([R, D], F32)
            nc.sync.dma_start(
                out=agin.rearrange("(n p) d -> p n d", p=128), in_=pns)
            agfull = dr.tile([M, D], F32, addr_space="Shared")
            nc.gpsimd.collective_compute(
                "AllGather", ALU.bypass,
                replica_groups=[list(range(NCORES))],
                ins=[agin], outs=[agfull])
            agpair = dr.tile([2 * R, D], F32)
            nc.gpsimd.collective_compute(
                "AllGather", ALU.bypass,
                replica_groups=[[k, k + 4] for k in range(4)],
                ins=[agin], outs=[agpair])

            # bf16 copy of own shard + transposes -> psT [64, R]
            pnsb = pns if not USE_BF16_MM else big.tile([128, NS, D], BF16)
            if USE_BF16_MM:
                nc.vector.tensor_copy(
                    pnsb.rearrange("p n d -> p (n d)"),
                    pns.rearrange("p n d -> p (n d)"))
            psT = big.tile([64, R], MMDT)
            for q4 in range(NS // 4):
                tpp = tp.tile([64, 512], MMDT, tag="tp")
                for q in range(4):
                    nn = 4 * q4 + q
                    nc.tensor.transpose(
                        tpp[:, q * 128:(q + 1) * 128], pnsb[:, nn, :],
                        identity)
                nc.vector.tensor_copy(psT[:, q4 * 512:(q4 + 1) * 512], tpp)

            # probs part (independent of the gathers; ACT stays on Exp)
            probs_t = big.tile([128, NS, NCLS], F32)
            nc.sync.dma_start(out=probs_t, in_=probs_d)
            tgt_t = big.tile([128, NS], F32)
            nc.sync.dma_start(out=tgt_t, in_=tgt_d)
            eprobs = big.tile([128, NS, NCLS], F32)
            nc.scalar.activation(
                eprobs.rearrange("p n c -> p (n c)"),
                probs_t.rearrange("p n c -> p (n c)"), AF.Exp)

            # ---- full pn columns: convert + transpose per group, with the
            # exp stream of earlier groups overlapping later group builds ----
            pnT = big.tile([64, M], MMDT)
            agv = agfull.rearrange("(n p) d -> p n d", p=128)
            scols = big.tile([128, NS * JJ], F32)
            for g in range(G):
                rawg = grp.tile([128, NTG, D], F32, tag="rawg")
                nc.sync.dma_start(
                    out=rawg, in_=agv[:, g * NTG:(g + 1) * NTG, :])
                if USE_BF16_MM:
                    png = grp.tile([128, NTG, D], BF16, tag="png")
                    nc.vector.tensor_copy(
                        png.rearrange("p n d -> p (n d)"),
                        rawg.rearrange("p n d -> p (n d)"))
                else:
                    png = rawg
                for t4 in range(NTG // 4):
                    tpp = tp.tile([64, 512], MMDT, tag="tp")
                    for q in range(4):
                        nn = 4 * t4 + q
                        nc.tensor.transpose(
                            tpp[:, q * 128:(q + 1) * 128], png[:, nn, :],
                            identity)
                    col = (g * NTG + t4 * 4) * 128
                    nc.vector.tensor_copy(pnT[:, col:col + 512], tpp)

                # main loop over this group's 1024-wide column blocks
                jlo = g * NTG * 128 // 1024
                jhi = (g + 1) * NTG * 128 // 1024
                for jj in range(jlo, jhi):
                    c0 = jj * 1024
                    for n in range(NS):
                        idx = n * JJ + jj
                        pst = mm.tile([128, 1024], F32, tag="mm")
                        lhsT = psT[:, n * 128:(n + 1) * 128]
                        nc.tensor.matmul(pst[:, 0:512], lhsT,
                                         pnT[:, c0:c0 + 512],
                                         start=True, stop=True)
                        nc.tensor.matmul(pst[:, 512:1024], lhsT,
                                         pnT[:, c0 + 512:c0 + 1024],
                                         start=True, stop=True)
                        et = esc.tile([128, 1024], F32, tag="esc")
                        if (idx * N_ACC) % (NS * JJ) < N_ACC:
                            nc.scalar.activation(
                                et, pst, AF.Exp, scale=INV_TEMP,
                                accum_out=scols[:, idx:idx + 1])
                        else:
                            nc.scalar.activation(et, pst, AF.Exp,
                                                 scale=INV_TEMP)
                            nc.vector.tensor_reduce(
                                scols[:, idx:idx + 1], et,
                                axis=mybir.AxisListType.X, op=ALU.add)

            sum10 = big.tile([128, NS], F32)
            nc.vector.tensor_reduce(sum10, eprobs, axis=mybir.AxisListType.X,
                                    op=ALU.add)
            own = big.tile([128, NS], F32)
            for n in range(NS):
                mask = work.tile([128, NCLS], F32, tag="mask")
                nc.vector.tensor_scalar(mask, iota10, tgt_t[:, n:n + 1], None,
                                        ALU.is_equal)
                nc.vector.tensor_mul(mask, mask, eprobs[:, n, :])
                nc.vector.tensor_reduce(own[:, n:n + 1], mask,
                                        axis=mybir.AxisListType.X, op=ALU.add)

            # pos_r and diag_r in fp32 — pair-gathered rows give
            # lo+hi = own + partner, so pos = pns.(lo+hi) - diag with static
            # addressing on every rank.  Emitted late so their DVE/ACT ops
            # cannot stall the main exp stream.
            prl = big.tile([128, NS, D], F32)
            nc.sync.dma_start(
                out=prl,
                in_=agpair[0:R, :].rearrange("(n p) d -> p n d", p=128))
            prh = big.tile([128, NS, D], F32)
            nc.sync.dma_start(
                out=prh,
                in_=agpair[R:2 * R, :].rearrange("(n p) d -> p n d", p=128))
            psum_rows = work.tile([128, NS, D], F32, tag="rowdot", bufs=2)
            nc.vector.tensor_add(psum_rows, prl, prh)

            diag_raw = big.tile([128, NS], F32)
            dsum_raw = big.tile([128, NS], F32)
            dq = work.tile([128, NS, D], F32, tag="rowdot", bufs=2)
            nc.vector.tensor_mul(dq, pns, pns)
            nc.vector.tensor_reduce(diag_raw, dq, axis=mybir.AxisListType.X,
                                    op=ALU.add)
            pq = work.tile([128, NS, D], F32, tag="rowdot", bufs=2)
            nc.vector.tensor_mul(pq, pns, psum_rows)
            nc.vector.tensor_reduce(dsum_raw, pq, axis=mybir.AxisListType.X,
                                    op=ALU.add)
            pos_raw = big.tile([128, NS], F32)
            nc.vector.tensor_sub(pos_raw, dsum_raw, diag_raw)

            ediag = big.tile([128, NS], F32)
            nc.scalar.activation(ediag, diag_raw, AF.Exp, scale=INV_TEMP)
            epos = big.tile([128, NS], F32)
            nc.scalar.activation(epos, pos_raw, AF.Exp, scale=INV_TEMP)
            pos2 = big.tile([128, NS], F32)
            nc.vector.tensor_scalar_mul(pos2, pos_raw, INV_TEMP)

            # ---- loss tails ----
            stot = big.tile([128, NS], F32)
            nc.vector.tensor_reduce(
                stot, scols.rearrange("p (n j) -> p n j", j=JJ),
                axis=mybir.AxisListType.X, op=ALU.add)
            s1 = big.tile([128, NS], F32)
            nc.vector.tensor_sub(s1, stot, ediag)
            lse1 = big.tile([128, NS], F32)
            nc.scalar.activation(lse1, s1, AF.Ln)
            c1 = big.tile([128, NS], F32)
            nc.vector.tensor_sub(c1, lse1, pos2)
            v12 = big.tile([128, 2], F32)
            nc.vector.tensor_reduce(v12[:, 0:1], c1,
                                    axis=mybir.AxisListType.X, op=ALU.add)

            s2 = big.tile([128, NS], F32)
            nc.vector.tensor_sub(s2, sum10, own)
            nc.vector.tensor_add(s2, s2, epos)
            # false data-dep on stot so the scheduler cannot hoist the Ln
            # into the exp stream (each hoist costs 2 ACT table swaps)
            nc.vector.scalar_tensor_tensor(
                out=s2, in0=stot, scalar=0.0, in1=s2,
                op0=ALU.mult, op1=ALU.add)
            lse2 = big.tile([128, NS], F32)
            nc.scalar.activation(lse2, s2, AF.Ln)
            c2 = big.tile([128, NS], F32)
            nc.vector.tensor_sub(c2, lse2, pos2)
            nc.vector.tensor_reduce(v12[:, 1:2], c2,
                                    axis=mybir.AxisListType.X, op=ALU.add)

            # ---- partition-sum via ones-matmul, then DMA out ----
            pso = po.tile([1, 2], F32)
            nc.tensor.matmul(pso, ones, v12, start=True, stop=True)
            outsb = big.tile([1, 2], F32)
            nc.vector.tensor_copy(outsb, pso)
            nc.sync.dma_start(out=out_d, in_=outsb)

    nc.compile()
    return nc


_NC_CACHE = None


def _get_nc():
    global _NC_CACHE
    if _NC_CACHE is None:
        _NC_CACHE = build_program()
    return _NC_CACHE


def make_in_maps(z_i, z_j, probs, target):
    p = np.ascontiguousarray(
        np.concatenate([z_i, z_j], axis=0), dtype=np.float32)
    t2 = np.concatenate([target, target]).astype(np.float32)
    probs = np.asarray(probs, dtype=np.float32)
    in_maps = []
    for k in range(NCORES):
        lo = k * R
        blob = np.concatenate([
            p[lo:lo + R].reshape(-1),
            probs[lo:lo + R].reshape(-1),
            t2[lo:lo + R],
        ]).reshape(1, -1)
        in_maps.append({"blob": np.ascontiguousarray(blob)})
    return in_maps


def kernel(z_i, z_j, probs, target, neg_idx):
    # neg_idx is the fixed structured NT-Xent mask (all columns except self and
    # positive); its effect is computed analytically, so it is never read.
    del neg_idx
    nc = _get_nc()
    in_maps = make_in_maps(np.asarray(z_i), np.asarray(z_j),
                           np.asarray(probs), np.asarray(target))
    res = run_bass_kernel_spmd(nc, in_maps, list(range(NCORES)))
    parts = np.stack([res.results[k]["out"].reshape(2) for k in range(NCORES)])
    total = parts.sum(axis=0) / np.float32(M)
    l1 = np.float32(total[0])
    l2 = np.float32(total[1])
    return (np.asarray(l1), np.asarray(l2))


# revision 10
# speedup vs baseline: 4.6441x; 1.2043x over previous
"""NT-Xent loss kernel for Trainium2, 8-core SPMD.

Math: with p = cat(z_i, z_j) [8192, 64], pn = p / max(||p||, 1e-8),
sim = 2 * pn @ pn.T (TEMP=0.5), the reference's gather-based losses reduce to
  loss1 = mean_r( log(sum_{c != r} exp(sim[r,c])) - pos_r )
  loss2 = mean_r( log(exp(pos_r) + sum_{c != t_r} exp(probs[r,c])) - pos_r )
where pos_r = sim[r, (r+N) % 2N].  sim entries lie in [-2, 2], so the exp
never overflows and no max-shift pass is needed.  The huge neg_idx input is a
fixed structured mask (drop self + positive) and never needs to be read.

Sharding: row-parallel.  Each of the 8 cores receives ONLY its own 1024 rows
of p (256 KB) plus its probs/target slices; it normalizes them locally, then
an on-device AllGather of the normalized rows builds the full [8192, 64] pn
every core needs for the sim columns.  The positive-pair rows pn[(r+N)%2N]
live on core k^4; a second, pairwise AllGather over groups {k, k+4} brings
them in, and pos is recovered order-independently as
  pos = pns . (lo + hi) - diag        (lo+hi = own + partner rows)
so the same SPMD program works on every rank with static addressing.
Each core emits two partial sums; the host adds the 8 partials.

Wall-clock notes (the metric here is warm dispatch wall time; the axon
NTFF-profile path is unavailable in this container):
  - inputs shrank 21.7 MB -> 2.4 MB per call (the dominant cost was host->
    device transfer over the axon tunnel),
  - the jax persistent compilation cache makes repeat dispatches skip the
    client-side walrus NEFF recompile (~140 ms/call),
  - iota is a NEFF-embedded Const tensor instead of a runtime input.
"""

import numpy as np

import jax

# Persistent compilation cache: run_bass_kernel_spmd builds a fresh jax.jit
# wrapper per call, so without this every dispatch re-runs the client-side
# walrus NEFF compile. With it, identical HLO (same BIR) is a disk cache hit.
try:
    jax.config.update("jax_compilation_cache_dir", "/tmp/jax_comp_cache_ntx")
    jax.config.update("jax_persistent_cache_min_compile_time_secs", 0)
    jax.config.update("jax_persistent_cache_min_entry_size_bytes", -1)
except Exception:
    pass

import concourse.bass as bass
import concourse.bacc as bacc
import concourse.tile as tile
from concourse import mybir
from concourse.masks import make_identity
from concourse.bass_utils import run_bass_kernel_spmd

N = 4096
D = 64
M = 2 * N            # 8192 rows of sim
NCORES = 8
R = M // NCORES      # 1024 rows per core
NT = M // 128        # 64 row-tiles of the full p
NS = R // 128        # 8 row-tiles of a shard
NCLS = 10
INV_TEMP = 2.0       # 1 / 0.5
F32 = mybir.dt.float32
BF16 = mybir.dt.bfloat16

# bf16 matmul for the sim slab: 4x PE throughput, 2x moving-dim. pos/diag
# stay fp32 (computed on DVE), and per-row errors average out over 8192 rows.
import os
USE_BF16_MM = os.environ.get("NTX_BF16", "1") == "1"
N_ACC_ENV = int(os.environ.get("NTX_NACC", "28"))
NEWTON_ITERS = int(os.environ.get("NTX_NEWT", "2"))

AF = mybir.ActivationFunctionType
ALU = mybir.AluOpType


def _emit_rsqrt(nc, pool, n2, nchunk, eng=None):
    """inv = 1/max(sqrt(n2), 1e-8), entirely on DVE: quake-style magic
    constant seed + Newton steps (ACT stays exclusively on Exp/Ln, so the
    activation table never thrashes)."""
    if eng is None:
        eng = nc.vector
    I32 = mybir.dt.int32
    inv = pool.tile([128, nchunk], F32, tag="rs_inv")
    eng.tensor_scalar(inv.bitcast(I32), n2.bitcast(I32), 1, None,
                      ALU.arith_shift_right)
    eng.tensor_scalar(inv.bitcast(I32), inv.bitcast(I32), -1, 0x5F3759DF,
                      ALU.mult, ALU.add)
    t2 = pool.tile([128, nchunk], F32, tag="rs_t2")
    for _ in range(NEWTON_ITERS):
        # y' = y * (1.5 - 0.5 * n2 * y^2)
        eng.tensor_mul(t2, inv, inv)
        eng.tensor_mul(t2, t2, n2)
        eng.tensor_scalar(t2, t2, -0.5, 1.5, ALU.mult, ALU.add)
        eng.tensor_mul(inv, inv, t2)
    eng.tensor_scalar_min(inv, inv, 1e8)
    return inv


def build_program():
    nc = bacc.Bacc("TRN2", target_bir_lowering=False, debug=False,
                   num_devices=NCORES)

    # One packed bf16 input per core: fewer h2d buffers per dispatch, half
    # the bytes.  bf16 z costs ~0.2% per-element quantization that averages
    # out over the 8192-row loss means (validated ~1e-5 final rel err).
    # Layout (bf16 elements): [0, R*D) zsh | [R*D, R*D+R*NCLS) probs |
    # [R*D+R*NCLS, R*D+R*NCLS+R) tgt.
    BLOB = R * D + R * NCLS + R
    blob_d = nc.dram_tensor("blob", [1, BLOB], BF16,
                            kind="ExternalInput").ap()
    zsh_d = blob_d[0, 0:R * D].rearrange("(n p d) -> p n d", p=128, d=D)
    probs_d = blob_d[0, R * D:R * D + R * NCLS].rearrange(
        "(n p c) -> p n c", p=128, c=NCLS)
    tgt_d = blob_d[0, R * D + R * NCLS:BLOB].rearrange("(n p) -> p n", p=128)
    out_d = nc.dram_tensor("out", [1, 2], F32, kind="ExternalOutput").ap()

    iota_h = nc.inline_tensor(
        np.broadcast_to(np.arange(NCLS, dtype=np.float32),
                        (128, NCLS)).copy(), name="iotah").ap()

    MMDT = BF16 if USE_BF16_MM else F32
    G = 4                 # pnT build groups over the gathered full p
    NTG = NT // G         # 16 row-chunks per group
    JJ = 8                # col groups of 1024 in the main loop
    N_ACC = N_ACC_ENV     # of the 64 row-sums, how many use ACT accum_out

    with tile.TileContext(nc) as tc:
        import contextlib
        with contextlib.ExitStack() as ctx:
            consts = ctx.enter_context(tc.tile_pool(name="consts", bufs=1))
            big = ctx.enter_context(tc.tile_pool(name="big", bufs=1))
            work = ctx.enter_context(tc.tile_pool(name="work", bufs=2))
            grp = ctx.enter_context(tc.tile_pool(name="grp", bufs=2))
            tp = ctx.enter_context(
                tc.tile_pool(name="tp", bufs=3, space="PSUM"))
            mm = ctx.enter_context(
                tc.tile_pool(name="mm", bufs=2, space="PSUM"))
            po = ctx.enter_context(
                tc.tile_pool(name="po", bufs=1, space="PSUM"))
            esc = ctx.enter_context(tc.tile_pool(name="esc", bufs=6))
            dr = ctx.enter_context(
                tc.tile_pool(name="dr", bufs=1, space="DRAM"))

            identity = consts.tile([128, 128], MMDT)
            make_identity(nc, identity)
            iota10 = consts.tile([128, NCLS], F32)
            nc.sync.dma_start(out=iota10, in_=iota_h)
            ones = consts.tile([128, 1], F32)
            nc.vector.memset(ones, 1.0)

            # ---- load + normalize this core's shard (fp32, DVE) ----
            rawsq_b = big.tile([128, NS, D], BF16)
            nc.sync.dma_start(out=rawsq_b, in_=zsh_d)
            rawsq = big.tile([128, NS, D], F32)
            nc.vector.tensor_copy(rawsq.rearrange("p n d -> p (n d)"),
                                  rawsq_b.rearrange("p n d -> p (n d)"))

            sflat = rawsq.rearrange("p n d -> p (n d)")
            s_sq = big.tile([128, NS * D], F32)
            s_n2 = big.tile([128, NS], F32)
            nc.vector.tensor_mul(s_sq, sflat, sflat)
            nc.vector.tensor_reduce(
                s_n2, s_sq.rearrange("p (n d) -> p n d", d=D),
                axis=mybir.AxisListType.X, op=ALU.add)
            s_inv = _emit_rsqrt(nc, big, s_n2, NS)
            pns = big.tile([128, NS, D], F32)
            for n in range(NS):
                nc.vector.tensor_scalar_mul(pns[:, n, :], rawsq[:, n, :],
                                            s_inv[:, n:n + 1])

            # ---- on-device gathers of the normalized rows ----
            agin = dr.tile([R, D], F32)
            nc.sync.dma_start(
                out=agin.rearrange("(n p) d -> p n d", p=128), in_=pns)
            agfull = dr.tile([M, D], F32, addr_space="Shared")
            nc.gpsimd.collective_compute(
                "AllGather", ALU.bypass,
                replica_groups=[list(range(NCORES))],
                ins=[agin], outs=[agfull])
            agpair = dr.tile([2 * R, D], F32)
            nc.gpsimd.collective_compute(
                "AllGather", ALU.bypass,
                replica_groups=[[k, k + 4] for k in range(4)],
                ins=[agin], outs=[agpair])

            # bf16 copy of own shard + transposes -> psT [64, R]
            pnsb = pns if not USE_BF16_MM else big.tile([128, NS, D], BF16)
            if USE_BF16_MM:
                nc.vector.tensor_copy(
                    pnsb.rearrange("p n d -> p (n d)"),
                    pns.rearrange("p n d -> p (n d)"))
            psT = big.tile([64, R], MMDT)
            for q4 in range(NS // 4):
                tpp = tp.tile([64, 512], MMDT, tag="tp")
                for q in range(4):
                    nn = 4 * q4 + q
                    nc.tensor.transpose(
                        tpp[:, q * 128:(q + 1) * 128], pnsb[:, nn, :],
                        identity)
                nc.vector.tensor_copy(psT[:, q4 * 512:(q4 + 1) * 512], tpp)

            # probs part (independent of the gathers; ACT stays on Exp)
            probs_b = big.tile([128, NS, NCLS], BF16)
            nc.sync.dma_start(out=probs_b, in_=probs_d)
            probs_t = big.tile([128, NS, NCLS], F32)
            nc.vector.tensor_copy(probs_t.rearrange("p n c -> p (n c)"),
                                  probs_b.rearrange("p n c -> p (n c)"))
            tgt_b = big.tile([128, NS], BF16)
            nc.sync.dma_start(out=tgt_b, in_=tgt_d)
            tgt_t = big.tile([128, NS], F32)
            nc.vector.tensor_copy(tgt_t, tgt_b)
            eprobs = big.tile([128, NS, NCLS], F32)
            nc.scalar.activation(
                eprobs.rearrange("p n c -> p (n c)"),
                probs_t.rearrange("p n c -> p (n c)"), AF.Exp)

            # ---- full pn columns: convert + transpose per group, with the
            # exp stream of earlier groups overlapping later group builds ----
            pnT = big.tile([64, M], MMDT)
            agv = agfull.rearrange("(n p) d -> p n d", p=128)
            scols = big.tile([128, NS * JJ], F32)
            for g in range(G):
                rawg = grp.tile([128, NTG, D], F32, tag="rawg")
                nc.sync.dma_start(
                    out=rawg, in_=agv[:, g * NTG:(g + 1) * NTG, :])
                if USE_BF16_MM:
                    png = grp.tile([128, NTG, D], BF16, tag="png")
                    nc.vector.tensor_copy(
                        png.rearrange("p n d -> p (n d)"),
                        rawg.rearrange("p n d -> p (n d)"))
                else:
                    png = rawg
                for t4 in range(NTG // 4):
                    tpp = tp.tile([64, 512], MMDT, tag="tp")
                    for q in range(4):
                        nn = 4 * t4 + q
                        nc.tensor.transpose(
                            tpp[:, q * 128:(q + 1) * 128], png[:, nn, :],
                            identity)
                    col = (g * NTG + t4 * 4) * 128
                    nc.vector.tensor_copy(pnT[:, col:col + 512], tpp)

                # main loop over this group's 1024-wide column blocks
                jlo = g * NTG * 128 // 1024
                jhi = (g + 1) * NTG * 128 // 1024
                for jj in range(jlo, jhi):
                    c0 = jj * 1024
                    for n in range(NS):
                        idx = n * JJ + jj
                        pst = mm.tile([128, 1024], F32, tag="mm")
                        lhsT = psT[:, n * 128:(n + 1) * 128]
                        nc.tensor.matmul(pst[:, 0:512], lhsT,
                                         pnT[:, c0:c0 + 512],
                                         start=True, stop=True)
                        nc.tensor.matmul(pst[:, 512:1024], lhsT,
                                         pnT[:, c0 + 512:c0 + 1024],
                                         start=True, stop=True)
                        et = esc.tile([128, 1024], F32, tag="esc")
                        if (idx * N_ACC) % (NS * JJ) < N_ACC:
                            nc.scalar.activation(
                                et, pst, AF.Exp, scale=INV_TEMP,
                                accum_out=scols[:, idx:idx + 1])
                        else:
                            nc.scalar.activation(et, pst, AF.Exp,
                                                 scale=INV_TEMP)
                            nc.vector.tensor_reduce(
                                scols[:, idx:idx + 1], et,
                                axis=mybir.AxisListType.X, op=ALU.add)

            sum10 = big.tile([128, NS], F32)
            nc.vector.tensor_reduce(sum10, eprobs, axis=mybir.AxisListType.X,
                                    op=ALU.add)
            own = big.tile([128, NS], F32)
            for n in range(NS):
                mask = work.tile([128, NCLS], F32, tag="mask")
                nc.vector.tensor_scalar(mask, iota10, tgt_t[:, n:n + 1], None,
                                        ALU.is_equal)
                nc.vector.tensor_mul(mask, mask, eprobs[:, n, :])
                nc.vector.tensor_reduce(own[:, n:n + 1], mask,
                                        axis=mybir.AxisListType.X, op=ALU.add)

            # pos_r and diag_r in fp32 — pair-gathered rows give
            # lo+hi = own + partner, so pos = pns.(lo+hi) - diag with static
            # addressing on every rank.  Emitted late so their DVE/ACT ops
            # cannot stall the main exp stream.
            prl = big.tile([128, NS, D], F32)
            nc.sync.dma_start(
                out=prl,
                in_=agpair[0:R, :].rearrange("(n p) d -> p n d", p=128))
            prh = big.tile([128, NS, D], F32)
            nc.sync.dma_start(
                out=prh,
                in_=agpair[R:2 * R, :].rearrange("(n p) d -> p n d", p=128))
            psum_rows = work.tile([128, NS, D], F32, tag="rowdot", bufs=2)
            nc.vector.tensor_add(psum_rows, prl, prh)

            diag_raw = big.tile([128, NS], F32)
            dsum_raw = big.tile([128, NS], F32)
            dq = work.tile([128, NS, D], F32, tag="rowdot", bufs=2)
            nc.vector.tensor_mul(dq, pns, pns)
            nc.vector.tensor_reduce(diag_raw, dq, axis=mybir.AxisListType.X,
                                    op=ALU.add)
            pq = work.tile([128, NS, D], F32, tag="rowdot", bufs=2)
            nc.vector.tensor_mul(pq, pns, psum_rows)
            nc.vector.tensor_reduce(dsum_raw, pq, axis=mybir.AxisListType.X,
                                    op=ALU.add)
            pos_raw = big.tile([128, NS], F32)
            nc.vector.tensor_sub(pos_raw, dsum_raw, diag_raw)

            ediag = big.tile([128, NS], F32)
            nc.scalar.activation(ediag, diag_raw, AF.Exp, scale=INV_TEMP)
            epos = big.tile([128, NS], F32)
            nc.scalar.activation(epos, pos_raw, AF.Exp, scale=INV_TEMP)
            pos2 = big.tile([128, NS], F32)
            nc.vector.tensor_scalar_mul(pos2, pos_raw, INV_TEMP)

            # ---- loss tails ----
            stot = big.tile([128, NS], F32)
            nc.vector.tensor_reduce(
                stot, scols.rearrange("p (n j) -> p n j", j=JJ),
                axis=mybir.AxisListType.X, op=ALU.add)
            s1 = big.tile([128, NS], F32)
            nc.vector.tensor_sub(s1, stot, ediag)
            lse1 = big.tile([128, NS], F32)
            nc.scalar.activation(lse1, s1, AF.Ln)
            c1 = big.tile([128, NS], F32)
            nc.vector.tensor_sub(c1, lse1, pos2)
            v12 = big.tile([128, 2], F32)
            nc.vector.tensor_reduce(v12[:, 0:1], c1,
                                    axis=mybir.AxisListType.X, op=ALU.add)

            s2 = big.tile([128, NS], F32)
            nc.vector.tensor_sub(s2, sum10, own)
            nc.vector.tensor_add(s2, s2, epos)
            # false data-dep on stot so the scheduler cannot hoist the Ln
            # into the exp stream (each hoist costs 2 ACT table swaps)
            nc.vector.scalar_tensor_tensor(
                out=s2, in0=stot, scalar=0.0, in1=s2,
                op0=ALU.mult, op1=ALU.add)
            lse2 = big.tile([128, NS], F32)
            nc.scalar.activation(lse2, s2, AF.Ln)
            c2 = big.tile([128, NS], F32)
            nc.vector.tensor_sub(c2, lse2, pos2)
            nc.vector.tensor_reduce(v12[:, 1:2], c2,
                                    axis=mybir.AxisListType.X, op=ALU.add)

            # ---- partition-sum via ones-matmul, then DMA out ----
            pso = po.tile([1, 2], F32)
            nc.tensor.matmul(pso, ones, v12, start=True, stop=True)
            outsb = big.tile([1, 2], F32)
            nc.vector.tensor_copy(outsb, pso)
            nc.sync.dma_start(out=out_d, in_=outsb)

    nc.compile()
    return nc


_NC_CACHE = None


def _get_nc():
    global _NC_CACHE
    if _NC_CACHE is None:
        _NC_CACHE = build_program()
    return _NC_CACHE


def make_in_maps(z_i, z_j, probs, target):
    p = np.ascontiguousarray(
        np.concatenate([z_i, z_j], axis=0), dtype=np.float32)
    t2 = np.concatenate([target, target]).astype(np.float32)
    probs = np.asarray(probs, dtype=np.float32)
    import ml_dtypes
    in_maps = []
    for k in range(NCORES):
        lo = k * R
        blob = np.concatenate([
            p[lo:lo + R].reshape(-1),
            probs[lo:lo + R].reshape(-1),
            t2[lo:lo + R],
        ]).reshape(1, -1).astype(ml_dtypes.bfloat16)
        in_maps.append({"blob": np.ascontiguousarray(blob)})
    return in_maps


def kernel(z_i, z_j, probs, target, neg_idx):
    # neg_idx is the fixed structured NT-Xent mask (all columns except self and
    # positive); its effect is computed analytically, so it is never read.
    del neg_idx
    nc = _get_nc()
    in_maps = make_in_maps(np.asarray(z_i), np.asarray(z_j),
                           np.asarray(probs), np.asarray(target))
    res = run_bass_kernel_spmd(nc, in_maps, list(range(NCORES)))
    parts = np.stack([res.results[k]["out"].reshape(2) for k in range(NCORES)])
    total = parts.sum(axis=0) / np.float32(M)
    l1 = np.float32(total[0])
    l2 = np.float32(total[1])
    return (np.asarray(l1), np.asarray(l2))


# revision 11
# speedup vs baseline: 5.6125x; 1.2085x over previous
"""NT-Xent loss kernel for Trainium2, 8-core SPMD.

Math: with p = cat(z_i, z_j) [8192, 64], pn = p / max(||p||, 1e-8),
sim = 2 * pn @ pn.T (TEMP=0.5), the reference's gather-based losses reduce to
  loss1 = mean_r( log(sum_{c != r} exp(sim[r,c])) - pos_r )
  loss2 = mean_r( log(exp(pos_r) + sum_{c != t_r} exp(probs[r,c])) - pos_r )
where pos_r = sim[r, (r+N) % 2N].  sim entries lie in [-2, 2], so the exp
never overflows and no max-shift pass is needed.  The huge neg_idx input is a
fixed structured mask (drop self + positive) and never needs to be read.

Sharding: row-parallel.  Each of the 8 cores receives ONLY its own 1024 rows
of p plus its probs / one-hot-target slices (one packed bf16 blob, ~172 KB);
it normalizes and transposes them locally, then an on-device AllGather of the
transposed bf16 blocks assembles the full [64, 8192] pnT every core needs for
the sim columns.  The positive-pair rows pn[(r+N)%2N] live on core k^4; a
second, pairwise AllGather over groups {k, k+4} brings them in, and pos is
recovered order-independently as
  pos = pns . (lo + hi) - diag        (lo+hi = own + partner rows)
so the same SPMD program works on every rank with static addressing.
Each core emits two partial sums; the host adds the 8 partials.

Wall-clock notes (the metric here is warm dispatch wall time; the axon
NTFF-profile path is unavailable in this container):
  - inputs shrank 21.7 MB -> 1.4 MB per call (host->device transfer over the
    axon tunnel was the dominant cost),
  - the jax persistent compilation cache makes repeat dispatches skip the
    client-side walrus NEFF recompile (~140 ms/call),
  - device work is minimized in instructions and elements: only the own
    shard is normalized/transposed per core, the one-hot target mask is
    precomputed on host, and the exp stream runs 32x [128, 2048] tiles with
    free ACT accum_out row-sums.
"""

import numpy as np

import jax

# Persistent compilation cache: run_bass_kernel_spmd builds a fresh jax.jit
# wrapper per call, so without this every dispatch re-runs the client-side
# walrus NEFF recompile. With it, identical HLO (same BIR) is a disk cache hit.
try:
    jax.config.update("jax_compilation_cache_dir", "/tmp/jax_comp_cache_ntx")
    jax.config.update("jax_persistent_cache_min_compile_time_secs", 0)
    jax.config.update("jax_persistent_cache_min_entry_size_bytes", -1)
except Exception:
    pass

import concourse.bass as bass
import concourse.bacc as bacc
import concourse.tile as tile
from concourse import mybir
from concourse.masks import make_identity
from concourse.bass_utils import run_bass_kernel_spmd

N = 4096
D = 64
M = 2 * N            # 8192 rows of sim
NCORES = 8
R = M // NCORES      # 1024 rows per core
NT = M // 128        # 64 row-tiles of the full p
NS = R // 128        # 8 row-tiles of a shard
NCLS = 10
INV_TEMP = 2.0       # 1 / 0.5
F32 = mybir.dt.float32
BF16 = mybir.dt.bfloat16

import os
# bf16 matmul for the sim slab: pos/diag stay fp32 (computed on DVE), and
# per-row errors average out over 8192 rows.
MM_W = int(os.environ.get("NTX_MMW", "512"))     # matmul free width
EXP_W = int(os.environ.get("NTX_EXPW", "2048"))  # exp tile width (PSUM)
NEWTON_ITERS = int(os.environ.get("NTX_NEWT", "2"))

AF = mybir.ActivationFunctionType
ALU = mybir.AluOpType


def _emit_rsqrt(nc, pool, n2, nchunk):
    """inv = 1/max(sqrt(n2), 1e-8), entirely on DVE: quake-style magic
    constant seed + Newton steps (ACT stays exclusively on Exp/Ln, so the
    activation table never thrashes)."""
    eng = nc.vector
    I32 = mybir.dt.int32
    inv = pool.tile([128, nchunk], F32, tag="rs_inv")
    eng.tensor_scalar(inv.bitcast(I32), n2.bitcast(I32), 1, None,
                      ALU.arith_shift_right)
    eng.tensor_scalar(inv.bitcast(I32), inv.bitcast(I32), -1, 0x5F3759DF,
                      ALU.mult, ALU.add)
    t2 = pool.tile([128, nchunk], F32, tag="rs_t2")
    for _ in range(NEWTON_ITERS):
        # y' = y * (1.5 - 0.5 * n2 * y^2)
        eng.tensor_mul(t2, inv, inv)
        eng.tensor_mul(t2, t2, n2)
        eng.tensor_scalar(t2, t2, -0.5, 1.5, ALU.mult, ALU.add)
        eng.tensor_mul(inv, inv, t2)
    eng.tensor_scalar_min(inv, inv, 1e8)
    return inv


def build_program():
    nc = bacc.Bacc("TRN2", target_bir_lowering=False, debug=False,
                   num_devices=NCORES)

    # One packed bf16 input per core: one h2d buffer per dispatch, half the
    # bytes of f32.  bf16 z costs ~0.2% per-element quantization that
    # averages out over the 8192-row loss means (validated ~1e-6 final).
    # Layout (bf16 elements):
    #   [0, R*D)                     zsh    — this core's rows of p
    #   [R*D, R*D+R*NCLS)            probs  — this core's probs rows
    #   [R*D+R*NCLS, R*D+2*R*NCLS)   onehot — (c == target_r) as 1.0/0.0
    BLOB = R * D + 2 * R * NCLS
    blob_d = nc.dram_tensor("blob", [1, BLOB], BF16,
                            kind="ExternalInput").ap()
    zsh_d = blob_d[0, 0:R * D].rearrange("(n p d) -> p n d", p=128, d=D)
    probs_d = blob_d[0, R * D:R * D + R * NCLS].rearrange(
        "(n p c) -> p n c", p=128, c=NCLS)
    oneh_d = blob_d[0, R * D + R * NCLS:BLOB].rearrange(
        "(n p c) -> p n c", p=128, c=NCLS)
    out_d = nc.dram_tensor("out", [1, 2], F32, kind="ExternalOutput").ap()

    JJ = M // EXP_W       # exp tiles per shard row-tile
    NMM = EXP_W // MM_W   # matmuls per exp tile

    with tile.TileContext(nc) as tc:
        import contextlib
        with contextlib.ExitStack() as ctx:
            consts = ctx.enter_context(tc.tile_pool(name="consts", bufs=1))
            big = ctx.enter_context(tc.tile_pool(name="big", bufs=1))
            work = ctx.enter_context(tc.tile_pool(name="work", bufs=2))
            tp = ctx.enter_context(
                tc.tile_pool(name="tp", bufs=3, space="PSUM"))
            mm = ctx.enter_context(
                tc.tile_pool(name="mm", bufs=1, space="PSUM"))
            po = ctx.enter_context(
                tc.tile_pool(name="po", bufs=1, space="PSUM"))
            esc = ctx.enter_context(tc.tile_pool(name="esc", bufs=4))
            dr = ctx.enter_context(
                tc.tile_pool(name="dr", bufs=1, space="DRAM"))

            identity = consts.tile([128, 128], BF16)
            make_identity(nc, identity)
            ones = consts.tile([128, 1], F32)
            nc.vector.memset(ones, 1.0)

            # ---- load + normalize this core's shard (fp32, DVE) ----
            rawsq_b = big.tile([128, NS, D], BF16)
            nc.sync.dma_start(out=rawsq_b, in_=zsh_d)
            rawsq = big.tile([128, NS, D], F32)
            nc.vector.tensor_copy(rawsq.rearrange("p n d -> p (n d)"),
                                  rawsq_b.rearrange("p n d -> p (n d)"))

            sflat = rawsq.rearrange("p n d -> p (n d)")
            s_sq = big.tile([128, NS * D], F32)
            s_n2 = big.tile([128, NS], F32)
            nc.vector.tensor_mul(s_sq, sflat, sflat)
            nc.vector.tensor_reduce(
                s_n2, s_sq.rearrange("p (n d) -> p n d", d=D),
                axis=mybir.AxisListType.X, op=ALU.add)
            s_inv = _emit_rsqrt(nc, big, s_n2, NS)
            pns = big.tile([128, NS, D], F32)
            for n in range(NS):
                nc.vector.tensor_scalar_mul(pns[:, n, :], rawsq[:, n, :],
                                            s_inv[:, n:n + 1])

            # bf16 copy + transposes -> psT [64, R] (lhsT for the slab, and
            # this core's contribution to the transposed all-gather)
            pnsb = big.tile([128, NS, D], BF16)
            nc.vector.tensor_copy(
                pnsb.rearrange("p n d -> p (n d)"),
                pns.rearrange("p n d -> p (n d)"))
            psT = big.tile([64, R], BF16)
            for q4 in range(NS // 4):
                tpp = tp.tile([64, 512], BF16, tag="tp")
                for q in range(4):
                    nn = 4 * q4 + q
                    nc.tensor.transpose(
                        tpp[:, q * 128:(q + 1) * 128], pnsb[:, nn, :],
                        identity)
                nc.vector.tensor_copy(psT[:, q4 * 512:(q4 + 1) * 512], tpp)

            # ---- on-device gathers ----
            # (1) transposed bf16 blocks -> pnT [64, 8192] via 8 block DMAs
            aginT = dr.tile([64, R], BF16)
            nc.sync.dma_start(out=aginT, in_=psT)
            agT = dr.tile([NCORES * 64, R], BF16, addr_space="Shared")
            nc.gpsimd.collective_compute(
                "AllGather", ALU.bypass,
                replica_groups=[list(range(NCORES))],
                ins=[aginT], outs=[agT])
            # (2) fp32 normalized rows of the pair {k, k+4} for pos
            agin = dr.tile([R, D], F32)
            nc.sync.dma_start(
                out=agin.rearrange("(n p) d -> p n d", p=128), in_=pns)
            agpair = dr.tile([2 * R, D], F32)
            nc.gpsimd.collective_compute(
                "AllGather", ALU.bypass,
                replica_groups=[[k, k + 4] for k in range(4)],
                ins=[agin], outs=[agpair])

            pnT = big.tile([64, M], BF16)
            for r in range(NCORES):
                nc.sync.dma_start(out=pnT[:, r * R:(r + 1) * R],
                                  in_=agT[r * 64:(r + 1) * 64, :])

            # probs part: exp on ACT, one-hot own-class sum from host mask
            probs_b = big.tile([128, NS, NCLS], BF16)
            nc.sync.dma_start(out=probs_b, in_=probs_d)
            probs_t = big.tile([128, NS, NCLS], F32)
            nc.vector.tensor_copy(probs_t.rearrange("p n c -> p (n c)"),
                                  probs_b.rearrange("p n c -> p (n c)"))
            oneh_b = big.tile([128, NS, NCLS], BF16)
            nc.sync.dma_start(out=oneh_b, in_=oneh_d)
            eprobs = big.tile([128, NS, NCLS], F32)
            nc.scalar.activation(
                eprobs.rearrange("p n c -> p (n c)"),
                probs_t.rearrange("p n c -> p (n c)"), AF.Exp)

            # ---- the sim slab: 8 row-tiles x JJ exp tiles of [128, EXP_W],
            # row sums via free ACT accum_out ----
            scols = big.tile([128, NS * JJ], F32)
            for n in range(NS):
                lhsT = psT[:, n * 128:(n + 1) * 128]
                for jj in range(JJ):
                    idx = n * JJ + jj
                    c0 = jj * EXP_W
                    pst = mm.tile([128, EXP_W], F32, tag="mm")
                    for q in range(NMM):
                        nc.tensor.matmul(
                            pst[:, q * MM_W:(q + 1) * MM_W], lhsT,
                            pnT[:, c0 + q * MM_W:c0 + (q + 1) * MM_W],
                            start=True, stop=True)
                    et = esc.tile([128, EXP_W], F32, tag="esc")
                    nc.scalar.activation(
                        et, pst, AF.Exp, scale=INV_TEMP,
                        accum_out=scols[:, idx:idx + 1])

            own = big.tile([128, NS], F32)
            omul = work.tile([128, NS, NCLS], F32, tag="omul")
            nc.vector.tensor_mul(
                omul.rearrange("p n c -> p (n c)"),
                eprobs.rearrange("p n c -> p (n c)"),
                oneh_b.rearrange("p n c -> p (n c)"))
            nc.vector.tensor_reduce(own, omul, axis=mybir.AxisListType.X,
                                    op=ALU.add)
            sum10 = big.tile([128, NS], F32)
            nc.vector.tensor_reduce(sum10, eprobs, axis=mybir.AxisListType.X,
                                    op=ALU.add)

            # pos_r and diag_r in fp32 — pair-gathered rows give
            # lo+hi = own + partner, so pos = pns.(lo+hi) - diag with static
            # addressing on every rank.  Emitted late so their DVE/ACT ops
            # cannot stall the main exp stream.
            prl = big.tile([128, NS, D], F32)
            nc.sync.dma_start(
                out=prl,
                in_=agpair[0:R, :].rearrange("(n p) d -> p n d", p=128))
            prh = big.tile([128, NS, D], F32)
            nc.sync.dma_start(
                out=prh,
                in_=agpair[R:2 * R, :].rearrange("(n p) d -> p n d", p=128))
            psum_rows = work.tile([128, NS, D], F32, tag="rowdot", bufs=2)
            nc.vector.tensor_add(psum_rows, prl, prh)

            diag_raw = big.tile([128, NS], F32)
            dsum_raw = big.tile([128, NS], F32)
            dq = work.tile([128, NS, D], F32, tag="rowdot", bufs=2)
            nc.vector.tensor_mul(dq, pns, pns)
            nc.vector.tensor_reduce(diag_raw, dq, axis=mybir.AxisListType.X,
                                    op=ALU.add)
            pq = work.tile([128, NS, D], F32, tag="rowdot", bufs=2)
            nc.vector.tensor_mul(pq, pns, psum_rows)
            nc.vector.tensor_reduce(dsum_raw, pq, axis=mybir.AxisListType.X,
                                    op=ALU.add)
            pos_raw = big.tile([128, NS], F32)
            nc.vector.tensor_sub(pos_raw, dsum_raw, diag_raw)

            ediag = big.tile([128, NS], F32)
            nc.scalar.activation(ediag, diag_raw, AF.Exp, scale=INV_TEMP)
            epos = big.tile([128, NS], F32)
            nc.scalar.activation(epos, pos_raw, AF.Exp, scale=INV_TEMP)
            pos2 = big.tile([128, NS], F32)
            nc.vector.tensor_scalar_mul(pos2, pos_raw, INV_TEMP)

            # ---- loss tails ----
            stot = big.tile([128, NS], F32)
            nc.vector.tensor_reduce(
                stot, scols.rearrange("p (n j) -> p n j", j=JJ),
                axis=mybir.AxisListType.X, op=ALU.add)
            s1 = big.tile([128, NS], F32)
            nc.vector.tensor_sub(s1, stot, ediag)
            lse1 = big.tile([128, NS], F32)
            nc.scalar.activation(lse1, s1, AF.Ln)
            c1 = big.tile([128, NS], F32)
            nc.vector.tensor_sub(c1, lse1, pos2)
            v12 = big.tile([128, 2], F32)
            nc.vector.tensor_reduce(v12[:, 0:1], c1,
                                    axis=mybir.AxisListType.X, op=ALU.add)

            s2 = big.tile([128, NS], F32)
            nc.vector.tensor_sub(s2, sum10, own)
            nc.vector.tensor_add(s2, s2, epos)
            # false data-dep on stot so the scheduler cannot hoist the Ln
            # into the exp stream (each hoist costs 2 ACT table swaps)
            nc.vector.scalar_tensor_tensor(
                out=s2, in0=stot, scalar=0.0, in1=s2,
                op0=ALU.mult, op1=ALU.add)
            lse2 = big.tile([128, NS], F32)
            nc.scalar.activation(lse2, s2, AF.Ln)
            c2 = big.tile([128, NS], F32)
            nc.vector.tensor_sub(c2, lse2, pos2)
            nc.vector.tensor_reduce(v12[:, 1:2], c2,
                                    axis=mybir.AxisListType.X, op=ALU.add)

            # ---- partition-sum via ones-matmul, then DMA out ----
            pso = po.tile([1, 2], F32)
            nc.tensor.matmul(pso, ones, v12, start=True, stop=True)
            outsb = big.tile([1, 2], F32)
            nc.vector.tensor_copy(outsb, pso)
            nc.sync.dma_start(out=out_d, in_=outsb)

    nc.compile()
    return nc


_NC_CACHE = None


def _get_nc():
    global _NC_CACHE
    if _NC_CACHE is None:
        _NC_CACHE = build_program()
    return _NC_CACHE


def make_in_maps(z_i, z_j, probs, target):
    import ml_dtypes
    p = np.ascontiguousarray(
        np.concatenate([z_i, z_j], axis=0), dtype=np.float32)
    t2 = np.concatenate([target, target]).astype(np.int64)
    probs = np.asarray(probs, dtype=np.float32)
    onehot = (np.arange(NCLS)[None, :] == t2[:, None]).astype(np.float32)
    in_maps = []
    for k in range(NCORES):
        lo = k * R
        blob = np.concatenate([
            p[lo:lo + R].reshape(-1),
            probs[lo:lo + R].reshape(-1),
            onehot[lo:lo + R].reshape(-1),
        ]).reshape(1, -1).astype(ml_dtypes.bfloat16)
        in_maps.append({"blob": np.ascontiguousarray(blob)})
    return in_maps


def kernel(z_i, z_j, probs, target, neg_idx):
    # neg_idx is the fixed structured NT-Xent mask (all columns except self and
    # positive); its effect is computed analytically, so it is never read.
    del neg_idx
    nc = _get_nc()
    in_maps = make_in_maps(np.asarray(z_i), np.asarray(z_j),
                           np.asarray(probs), np.asarray(target))
    res = run_bass_kernel_spmd(nc, in_maps, list(range(NCORES)))
    parts = np.stack([res.results[k]["out"].reshape(2) for k in range(NCORES)])
    total = parts.sum(axis=0) / np.float32(M)
    l1 = np.float32(total[0])
    l2 = np.float32(total[1])
    return (np.asarray(l1), np.asarray(l2))
